# revision 14
# baseline (speedup 1.0000x reference)
"""Single-head causal attention on 8 TRN2 NeuronCores.

Problem shapes (hardcoded): B=8, T=2048, C=1024, H=64, fp32 I/O.
    q = x @ Wq; k = x @ Wk; v = x @ Wv          (per batch element)
    wei = softmax(causal_mask(q @ k.T * C**-0.5))
    out = wei @ v
Sharding: pure data parallel - one batch element per core, no collectives.

Per-core algorithm (fp8 DoubleRow projections, bf16 attention, fp32 PSUM):
  - host quantizes xT to e4m3 (x8) plus an e4m3 residual r8 = xT - x8, and
    weights to e4m3 at folded scales (Wqk*16, Wv*32) with e4m3 weight
    residuals.  QKV projections run as fp8 DoubleRow matmuls (2 C-rows per
    partition, 2 cols/cycle):
      qk' = x8 @ (Wqk8 + Wqkr8)               (= 16*[q|k] + O(0.03%))
      v'  = x8 @ (Wv8 + Wvr8) + r8 @ Wv8      (= 32*v    + O(0.2%))
    fp8 input stream is 2 MB instead of 4 MB bf16, halving the head DMA.
  - S^T row-packed as in the bf16 version: kT2 pairs in partition halves,
    q duplicated into the hi half (qT2hi); h0 reads q straight from the
    projection cast (qkT rows 0:64).  exp scale folds the 1/256:
    P = exp(S' / 8192).  Causal diag blocks masked by a 0/1 multiply.
  - v1 = [32*v | 32.0] so [num|den] share one accumulator and the 32x
    scale cancels in the normalize (rc = 1/(32 den), ot = 32 num * rc).
  - per-slice pipeline: fp8 projections -> DVE cast + shuffle DMAs ->
    S^T pair tiles feed ScalarE exp (the secondary bottleneck, ~17us);
    AV matmuls trail exp by one tile; v-finish (r8 term + transpose back)
    deferred until r8 lands; epilogue deferred one slice.
  - input DMAs split across the two hardware DGE queues (sync/scalar) in
    T-half chunks so slice 0 unblocks after ~3us; shuffles ride the
    software queues (gpsimd/vector); casts split between DVE and Pool.
"""

import numpy as np
import ml_dtypes

import concourse.bass as bass
import concourse.mybir as mybir
import concourse.tile as tile
from concourse import bacc
from concourse.bass_utils import run_bass_kernel_spmd

B, T, C, H = 8, 2048, 1024, 64
NT = T // 128           # 16 Tk-blocks of 128
NJ = T // 512           # 4 Tq-slices of 512
SCALE = (float(C) ** -0.5) / 256.0  # exp scale with 16x-weight fold

BF16 = mybir.dt.bfloat16
F32 = mybir.dt.float32
F8 = mybir.dt.float8e4
DR = mybir.MatmulPerfMode.DoubleRow
npbf16 = ml_dtypes.bfloat16
npf8 = ml_dtypes.float8_e4m3fn


def build_attention(nc: bass.Bass, tc: tile.TileContext, ctx):
    x8_d = nc.dram_tensor("x8", [128, 8, T], F8, kind="ExternalInput").ap()
    r8_d = nc.dram_tensor("r8", [128, 8, T], F8, kind="ExternalInput").ap()
    wqk8_d = nc.dram_tensor("wqk8", [128, 4, 2, 128], F8,
                            kind="ExternalInput").ap()
    wqkr8_d = nc.dram_tensor("wqkr8", [128, 4, 2, 128], F8,
                             kind="ExternalInput").ap()
    wv8_d = nc.dram_tensor("wv8", [128, 4, 2, H], F8,
                           kind="ExternalInput").ap()
    wvr8_d = nc.dram_tensor("wvr8", [128, 4, 2, H], F8,
                            kind="ExternalInput").ap()
    out_d = nc.dram_tensor("out", [T, H], F32, kind="ExternalOutput").ap()

    i64_2_np = np.concatenate([np.eye(64, dtype=npbf16)] * 2, axis=0)
    ident64_2 = nc.inline_tensor(i64_2_np, name="ident64_2").ap()
    ident65 = nc.inline_tensor(np.eye(65, dtype=npbf16), name="ident65").ap()
    causal_np = np.triu(np.ones((128, 128), dtype=npbf16))  # keep Tk<=Tq
    causal_d = nc.inline_tensor(causal_np, name="causal").ap()

    consts = ctx.enter_context(tc.tile_pool(name="consts", bufs=1))
    persist = ctx.enter_context(tc.tile_pool(name="persist", bufs=1))
    pts = ctx.enter_context(tc.tile_pool(name="pts", bufs=6))
    outts = ctx.enter_context(tc.tile_pool(name="outts", bufs=2))
    outs = ctx.enter_context(tc.tile_pool(name="outs", bufs=16))
    smalls = ctx.enter_context(tc.tile_pool(name="smalls", bufs=2))
    ps_big = ctx.enter_context(tc.tile_pool(name="ps_big", bufs=2,
                                            space="PSUM"))
    ps_v = ctx.enter_context(tc.tile_pool(name="ps_v", bufs=2, space="PSUM"))
    ps_mix = ctx.enter_context(tc.tile_pool(name="ps_mix", bufs=2,
                                            space="PSUM"))

    # Input DMAs.  scalar + sync are the two hardware DGE queues; emit the
    # first-needed tensors first, split by T-half so slice 0 unblocks early.
    wqk8_sb = consts.tile([128, 4, 2, 128], F8, tag="wqk8")
    nc.scalar.dma_start(out=wqk8_sb, in_=wqk8_d)
    wqkr8_sb = consts.tile([128, 4, 2, 128], F8, tag="wqkr8")
    nc.scalar.dma_start(out=wqkr8_sb, in_=wqkr8_d)

    wv8_sb = consts.tile([128, 4, 2, H], F8, tag="wv8")
    nc.sync.dma_start(out=wv8_sb, in_=wv8_d)
    wvr8_sb = consts.tile([128, 4, 2, H], F8, tag="wvr8")
    nc.sync.dma_start(out=wvr8_sb, in_=wvr8_d)
    i64_sb = consts.tile([128, 64], BF16, tag="i64")
    nc.sync.dma_start(out=i64_sb, in_=ident64_2)
    i65_sb = consts.tile([65, 65], BF16, tag="i65")
    nc.sync.dma_start(out=i65_sb, in_=ident65)
    causal_sb = consts.tile([128, 128], BF16, tag="causal")
    nc.sync.dma_start(out=causal_sb, in_=causal_d)

    x8_sb = persist.tile([128, 8, T], F8, tag="x8")
    r8_sb = persist.tile([128, 8, T], F8, tag="r8")
    half = T // 2
    for h0 in (0, half):
        hs = slice(h0, h0 + half)
        nc.sync.dma_start(out=x8_sb[:, 0:4, hs], in_=x8_d[:, 0:4, hs])
        nc.scalar.dma_start(out=x8_sb[:, 4:8, hs], in_=x8_d[:, 4:8, hs])
    for h0 in (0, half):
        hs = slice(h0, h0 + half)
        nc.scalar.dma_start(out=r8_sb[:, 4:8, hs], in_=r8_d[:, 4:8, hs])
    # r8 low c-half rides sync but is emitted inside the j-loop so the
    # slice-j shuffle descriptors outrank it in the queue FIFO.

    qkT = persist.tile([128, T], BF16, tag="qkT")     # [16q; 16k]
    qT2hi = persist.tile([128, T], BF16, tag="qT2hi")  # 16q in rows 64:128
    kT2 = persist.tile([128, T // 2], BF16, tag="kT2")  # Tk pairs in halves
    vT = persist.tile([64, T], BF16, tag="vT")          # 32*v^T
    vT2 = persist.tile([128, T // 2], BF16, tag="vT2")  # odd Tk blocks, hi
    v1 = persist.tile([128, NT, H + 1], BF16, tag="v1")  # [32v | 32]
    nc.vector.memset(v1, 32.0)

    pending_av = None
    stores = []  # (ot tile, row0) deferred so stores never stall the sync q
    for j in range(NJ):
        jsl = slice(j * 512, (j + 1) * 512)

        # ---- fp8 DoubleRow projections for slice j --------------------
        qk_ps = ps_mix.tile([128, 512], F32, tag="mix", name=f"qk_ps{j}")
        for b in range(4):
            nc.tensor.matmul(qk_ps, lhsT=wqk8_sb[:, b, :, :],
                             rhs=x8_sb[:, 2 * b:2 * b + 2, jsl],
                             start=(b == 0), stop=False, perf_mode=DR)
        for b in range(4):
            nc.tensor.matmul(qk_ps, lhsT=wqkr8_sb[:, b, :, :],
                             rhs=x8_sb[:, 2 * b:2 * b + 2, jsl],
                             start=False, stop=(b == 3), perf_mode=DR)
        nc.vector.tensor_copy(qkT[:, jsl], qk_ps)
        # odd k-blocks (4j+1, 4j+3) straight from PSUM into kT2 hi half
        for b in (1, 3):
            c0 = (2 * j + b // 2) * 128
            nc.vector.tensor_copy(kT2[64:128, c0:c0 + 128],
                                  qk_ps[64:128, b * 128:(b + 1) * 128])
        # q into hi half; even k-blocks into kT2 lo half (partition shift);
        # sync is a hardware DGE queue so these land fast
        nc.sync.dma_start(out=qT2hi[64:128, jsl], in_=qkT[0:64, jsl])
        for b in (0, 2):
            c0 = (2 * j + b // 2) * 128
            nc.sync.dma_start(
                out=kT2[0:64, c0:c0 + 128],
                in_=qkT[64:128, j * 512 + b * 128:j * 512 + (b + 1) * 128])
        if j < 2:  # r8 low c-half for T-half j, behind slice-j shuffles
            hs = slice(j * half, (j + 1) * half)
            nc.sync.dma_start(out=r8_sb[:, 0:4, hs], in_=r8_d[:, 0:4, hs])

        v_ps = ps_v.tile([128, 512], F32, tag="vps", name=f"v_ps{j}")
        for b in range(4):
            nc.tensor.matmul(v_ps[0:64, :], lhsT=wv8_sb[:, b, :, :],
                             rhs=x8_sb[:, 2 * b:2 * b + 2, jsl],
                             start=(b == 0), stop=False, perf_mode=DR)
        for b in range(4):
            nc.tensor.matmul(v_ps[0:64, :], lhsT=wvr8_sb[:, b, :, :],
                             rhs=x8_sb[:, 2 * b:2 * b + 2, jsl],
                             start=False, stop=False, perf_mode=DR)
        # r8 @ Wv8 term deferred to emit_vfinish (r8 lands later)

        # ---- deferred epilogue of slice j-1 ---------------------------
        if pending_av is not None:
            emit_epilogue(nc, outts, outs, smalls, ps_mix, i65_sb, stores,
                          *pending_av)
            pending_av = None

        # ---- v-finish for slice j (r8 term, cast, transpose to v1) ----
        emit_vfinish(nc, ps_mix, wv8_sb, r8_sb, v_ps, vT, vT2, v1, i64_sb, j)

        # ---- attention for slice j (row-packed S^T, pipelined AV) -----
        av = ps_mix.tile([65, 512], F32, tag="mix", name=f"av{j}")
        nblk = 4 * j + 4
        prev = None
        for m in range(2 * j + 2):
            sp2 = ps_big.tile([128, 1024], F32, tag="big", name=f"sp{j}_{m}")
            pt2 = pts.tile([128, 1024], BF16, tag="pt", name=f"pt{j}_{m}")
            n0s = []
            for half_idx, i in ((0, 2 * m), (1, 2 * m + 1)):
                g = i - 4 * j
                n0 = max(0, g) * 128
                p0 = half_idx * 64
                o = half_idx * 512
                rhs = (qkT if half_idx == 0 else qT2hi)
                nc.tensor.matmul(
                    sp2[:, o + n0:o + 512],
                    lhsT=kT2[p0:p0 + 64, m * 128:(m + 1) * 128],
                    rhs=rhs[p0:p0 + 64, j * 512 + n0:(j + 1) * 512],
                    start=True, stop=True)
                n0s.append(n0)
            if n0s[0] == 0 and n0s[1] == 0:  # one wide exp over both banks
                nc.scalar.activation(pt2, sp2,
                                     mybir.ActivationFunctionType.Exp,
                                     scale=SCALE)
            else:
                for half_idx in range(2):
                    o, n0 = half_idx * 512, n0s[half_idx]
                    nc.scalar.activation(
                        pt2[:, o + n0:o + 512], sp2[:, o + n0:o + 512],
                        mybir.ActivationFunctionType.Exp, scale=SCALE)
            for half_idx, i in ((0, 2 * m), (1, 2 * m + 1)):
                g = i - 4 * j
                if g >= 0:  # mask upper triangle of the diagonal block
                    o = half_idx * 512 + n0s[half_idx]
                    nc.gpsimd.tensor_mul(
                        pt2[:, o:o + 128], pt2[:, o:o + 128], causal_sb)
            if prev is not None:
                emit_av(nc, av, v1, *prev, nblk)
            prev = (pt2, n0s, 2 * m)
        emit_av(nc, av, v1, *prev, nblk)
        pending_av = (av, j)

    emit_epilogue(nc, outts, outs, smalls, ps_mix, i65_sb, stores,
                  *pending_av)
    for ot, r0 in stores:
        nc.sync.dma_start(out=out_d[r0:r0 + 128, :], in_=ot)


def emit_vfinish(nc, ps_mix, wv8_sb, r8_sb, v_ps, vT, vT2, v1, i64_sb, j):
    """r8 @ Wv8 correction, 32v^T cast, and transpose back into v1."""
    jsl = slice(j * 512, (j + 1) * 512)
    for b in range(4):
        nc.tensor.matmul(v_ps[0:64, :], lhsT=wv8_sb[:, b, :, :],
                         rhs=r8_sb[:, 2 * b:2 * b + 2, jsl],
                         start=False, stop=(b == 3), perf_mode=DR)
    nc.vector.tensor_copy(vT[:, jsl], v_ps[0:64, :])
    # odd Tk blocks 4j+1, 4j+3 -> vT2 hi half (partition shift via DMA)
    for bb in range(2):
        tb = 4 * j + 2 * bb + 1
        c0 = (2 * j + bb) * 128
        nc.gpsimd.dma_start(out=vT2[64:128, c0:c0 + 128],
                            in_=vT[:, tb * 128:(tb + 1) * 128])
    # v natural via row-packed identity matmuls (pair of Tk blocks)
    for mt in (2 * j, 2 * j + 1):
        tA, tB = 2 * mt, 2 * mt + 1
        vpA = ps_mix.tile([128, H], F32, tag="mix", name=f"vpA{mt}")
        vpB = ps_mix.tile([128, H], F32, tag="mix", name=f"vpB{mt}")
        nc.tensor.matmul(vpA, lhsT=vT[:, tA * 128:(tA + 1) * 128],
                         rhs=i64_sb[0:64, :], start=True, stop=True)
        nc.tensor.matmul(vpB, lhsT=vT2[64:128, mt * 128:(mt + 1) * 128],
                         rhs=i64_sb[64:128, :], start=True, stop=True)
        nc.vector.tensor_copy(v1[:, tA, 0:H], vpA)
        nc.vector.tensor_copy(v1[:, tB, 0:H], vpB)


def emit_av(nc, av, v1, pt2, n0s, i0, nblk):
    for d in range(2):
        i = i0 + d
        o, n0 = d * 512, n0s[d]
        nc.tensor.matmul(av[:, n0:512], lhsT=v1[:, i, :],
                         rhs=pt2[:, o + n0:o + 512],
                         start=(i == 0), stop=(i == nblk - 1))


def emit_epilogue(nc, outts, outs, smalls, ps_mix, i65_sb, stores, av, j):
    osb = outts.tile([65, 512], BF16, tag="osb", name=f"osb{j}")
    nc.vector.tensor_copy(osb, av)  # f32 PSUM -> bf16 SBUF
    for t in range(4):
        op = ps_mix.tile([128, H + 1], F32, tag="mix", name=f"op{j}_{t}")
        nc.tensor.matmul(op, lhsT=osb[:, t * 128:(t + 1) * 128], rhs=i65_sb,
                         start=True, stop=True)
        rc = smalls.tile([128, 1], F32, tag="rc", name=f"rc{j}_{t}")
        nc.vector.reciprocal(rc, op[:, H:H + 1])  # = 1/(32 den)
        ot = outs.tile([128, H], F32, tag="ot", name=f"ot{j}_{t}")
        nc.vector.tensor_scalar_mul(ot, op[:, 0:H], rc)  # 32num/(32den)
        stores.append((ot, (j * 4 + t) * 128))


_CACHED = {}


def _get_nc():
    if "nc" not in _CACHED:
        from contextlib import ExitStack
        nc = bacc.Bacc("TRN2", target_bir_lowering=False, debug=False,
                       num_devices=B)
        with tile.TileContext(nc) as tc:
            with ExitStack() as ctx:
                build_attention(nc, tc, ctx)
        nc.compile()
        _CACHED["nc"] = nc
    return _CACHED["nc"]


def _quant_inputs(inputs, Wq, Wk, Wv):
    """Host-side fp8 prep: per-batch x8/r8 in [128, 8, T] layout, weights
    at folded scales with e4m3 residuals in [128, 4, 2, M] layout."""
    inputs = np.asarray(inputs, dtype=np.float32)

    def wlayout(w, m):  # [C, m] -> [128, 4, 2, m]
        return np.ascontiguousarray(
            w.reshape(4, 2, 128, m).transpose(2, 0, 1, 3))

    wqk = np.concatenate([np.asarray(Wq), np.asarray(Wk)], axis=1)
    wqk = wqk.astype(np.float32) * 16.0
    wqk8 = wqk.astype(npf8)
    wqkr8 = (wqk - wqk8.astype(np.float32)).astype(npf8)
    wv = np.asarray(Wv).astype(np.float32) * 32.0
    wv8 = wv.astype(npf8)
    wvr8 = (wv - wv8.astype(np.float32)).astype(npf8)
    wqk8, wqkr8 = wlayout(wqk8, 128), wlayout(wqkr8, 128)
    wv8, wvr8 = wlayout(wv8, H), wlayout(wvr8, H)

    in_maps = []
    for b in range(B):
        xT = inputs[b].T  # [C, T] fp32
        x8 = xT.astype(npf8)
        r8 = (xT - x8.astype(np.float32)).astype(npf8)
        x8 = np.ascontiguousarray(x8.reshape(8, 128, T).transpose(1, 0, 2))
        r8 = np.ascontiguousarray(r8.reshape(8, 128, T).transpose(1, 0, 2))
        in_maps.append({"x8": x8, "r8": r8, "wqk8": wqk8, "wqkr8": wqkr8,
                        "wv8": wv8, "wvr8": wvr8})
    return in_maps


def kernel(inputs, Wq, Wk, Wv):
    in_maps = _quant_inputs(inputs, Wq, Wk, Wv)
    nc = _get_nc()
    res = run_bass_kernel_spmd(nc, in_maps, core_ids=list(range(B)))
    out = np.stack([res.results[b]["out"] for b in range(B)], axis=0)
    return out.astype(np.float32)
